# revision 18
# baseline (speedup 1.0000x reference)
"""Trainium2 Bass kernel for nn_CrossAttention (dense_transformer).

Reference computation (per batch b, per stream s in {1,2}):
    q_s   = heads(x_s)                      # [H, N, D] slices of x_s
    kv_s  = x_s @ Wkv_s -> k_s, v_s         # [N, C] each
    gate_s= sigmoid(relu(x_s @ w1 + b1) @ w2 + b2)
    ctx_s = softmax_d( scale * k_s^T @ (v_s * gate_s) )   # [H, D, D]
    o_1   = q_1 @ ctx_2 ; o_2 = q_2 @ ctx_1  (cross)

Sharding: 8 cores = (stream s, batch b) pairs; core (s, b) projects
x_s[b] and computes o_{1-s}[b] = q_{1-s}[b] @ softmax(ctx_s[b]).
No cross-core communication; host concatenates outputs.

v3 design (all-bf16 matmul path):
  * Host pre-transposes/casts x -> xT bf16: no PE transposes, no DRAM
    spills; weights cast to bf16 on host.
  * Fused per-1024-row super-chunk pipeline: gate1 (w1-stationary,
    xT streaming) -> hT; gate2 (hT-stationary, w2 streaming) -> g;
    kv (xT-stationary, wkv streaming) -> k, vg = v*g (DVE from PSUM);
    ctxT accumulated directly in PSUM across all 32 n-chunks.
  * xT streams through a rolling 2-super-chunk buffer; all of xqT is
    prefetched during phase A so phase B only has output DMA traffic.
  * Startup DMAs are split so the first matmul issues ~4us in; a
    keep-warm matmul inside the softmax chain stops HAM re-throttle.
  * Output phase holds block-diag softmax pairs stationary and streams
    xqT, producing oT bf16; the host transposes/casts back.
"""

import numpy as np
from contextlib import ExitStack

N = 4096
C = 1024
H = 16
D = 64
SCALE = D ** (-0.5)
NCH = N // 128       # 32 n-chunks of 128 rows
SC = 4               # super-chunks of 1024 rows
NPS = N // SC        # 1024 rows per super-chunk

_CACHE = {}


def _build_program(with_bias):
    """Build the SPMD Bass program (same for all 8 cores)."""
    import concourse.bass as bass
    import concourse.bacc as bacc
    import concourse.tile as tile
    import concourse.mybir as mybir

    F32 = mybir.dt.float32
    BF16 = mybir.dt.bfloat16
    AF = mybir.ActivationFunctionType

    nc = bacc.Bacc("TRN2", target_bir_lowering=False, debug=False, num_devices=8)

    xpt = nc.dram_tensor("xpt", [SC * 128, 8, NPS], BF16, kind="ExternalInput").ap()
    xqt = nc.dram_tensor("xqt", [C, N], BF16, kind="ExternalInput").ap()
    wkv = nc.dram_tensor("wkv", [128, 8, 2 * C], BF16, kind="ExternalInput").ap()
    w1 = nc.dram_tensor("w1", [128, 64, 128], BF16, kind="ExternalInput").ap()
    b1 = nc.dram_tensor("b1", [C], F32, kind="ExternalInput").ap()
    w2 = nc.dram_tensor("w2", [128, 8, C], BF16, kind="ExternalInput").ap()
    b2 = nc.dram_tensor("b2", [C], BF16, kind="ExternalInput").ap()
    ident = nc.dram_tensor("ident", [128, 128], F32, kind="ExternalInput").ap()
    o = nc.dram_tensor("o", [C, N], BF16, kind="ExternalOutput").ap()

    with tile.TileContext(nc) as tc, ExitStack() as ctx:
        # ---------- persistent pools ----------
        cpool = ctx.enter_context(tc.tile_pool(name="consts", bufs=1))
        ident_sb = cpool.tile([128, 128], F32, name="ident_sb")
        b1_sb = cpool.tile([128, 8], F32, name="b1_sb")  # b1_sb[p, m] = b1[m*128+p]
        if with_bias:
            ones_sb = cpool.tile([1, 128], BF16, name="ones_sb")
            nc.vector.memset(ones_sb, 1.0)
            b2_sb = cpool.tile([1, C], BF16, name="b2_sb")
            nc.sync.dma_start(b2_sb, b2.rearrange("(one f) -> one f", one=1))

        spool = ctx.enter_context(tc.tile_pool(name="spairs", bufs=1))
        spairs = [spool.tile([128, 128], BF16, name=f"spair{j}") for j in range(8)]
        st = spool.tile([64, 1024], F32, name="st")
        # off-diagonal blocks of the spairs are zero for the whole run;
        # zero them once up front while the DVE is otherwise idle
        for j in range(8):
            nc.vector.memset(spairs[j], 0.0)

        # all of xqT is prefetched during phase A so phase B is not
        # DMA-bound at the tail of the kernel
        xq_pool = ctx.enter_context(tc.tile_pool(name="bxq", bufs=1))
        xq_sb = [
            xq_pool.tile([128, N], BF16, name=f"xq{j}") for j in range(8)
        ]

        # =========================================================
        # Phase A: fused gate MLP + kv + ctx accumulation.
        # =========================================================
        with ExitStack() as pa:
            wpool = pa.enter_context(tc.tile_pool(name="weights", bufs=1))
            # w1_sb[p, m*8+k, m'] = w1[k*128+p, m*128+m'] (m-chunk major so
            # the first m-chunk is one small contiguous DMA)
            w1_sb = wpool.tile([128, 64, 128], BF16, name="w1_sb")
            w2_sb = wpool.tile([128, 8, C], BF16, name="w2_sb")   # [p, m, t]
            wkv_sb = wpool.tile([128, 8, 2 * C], BF16, name="wkv_sb")

            xpool = pa.enter_context(tc.tile_pool(name="xpt", bufs=2))
            xpt_t = [
                xpool.tile([128, 8, NPS], BF16, name="xpt_t", tag="xpt_t")
                for _ in range(SC)
            ]

            # Startup DMAs split over three dispatch queues so the first
            # matmul's inputs (w1 m-chunk 0 + first xT half) arrive ASAP:
            # sync (HWDGE) carries the critical compute-ordered stream,
            # scalar (HWDGE) the small constants + w2, gpsimd (SWDGE) the
            # xqT prefetches that are only needed in phase B.
            nc.scalar.dma_start(ident_sb, ident)
            nc.scalar.dma_start(b1_sb, b1.rearrange("(m p) -> p m", p=128))
            nc.sync.dma_start(w1_sb[:, 0:8, :], w1[:, 0:8, :])
            nc.sync.dma_start(xpt_t[0], xpt[0:128])
            nc.sync.dma_start(w1_sb[:, 8:64, :], w1[:, 8:64, :])
            nc.scalar.dma_start(w2_sb, w2)
            nc.sync.dma_start(wkv_sb[:, :, 0:C], wkv[:, :, 0:C])
            nc.sync.dma_start(wkv_sb[:, :, C:2 * C], wkv[:, :, C:2 * C])
            nc.sync.dma_start(xpt_t[1], xpt[128:256])
            for j in range(8):
                nc.sync.dma_start(xq_sb[j], xqt[j * 128:(j + 1) * 128, :])
            # xpt_t[2]/[3] alias the sc0/sc1 buffers; their refill DMAs are
            # emitted inside the sc loop AFTER the aliased buffer's last
            # reader so the dependency tracker orders them correctly.

            ctxps_pool = pa.enter_context(
                tc.tile_pool(name="ctxps", bufs=1, space="PSUM")
            )
            # ctxT accumulator: head h -> cols [h*64,(h+1)*64), layout [e, d]
            ctx_ps = ctxps_pool.tile([64, 1024], F32, name="ctx_ps")

            ps_pool = pa.enter_context(
                tc.tile_pool(name="psproj", bufs=5, space="PSUM")
            )
            h_pool = pa.enter_context(tc.tile_pool(name="hT", bufs=1))
            g_pool = pa.enter_context(tc.tile_pool(name="g", bufs=3))
            k_pool = pa.enter_context(tc.tile_pool(name="k", bufs=2))
            vg_pool = pa.enter_context(tc.tile_pool(name="vg", bufs=2))

            pending_ctx = []  # (k_sb, vg, nch) awaiting ctx accumulation

            def emit_ctx(last):
                k_sb, vg, nch = pending_ctx.pop(0)
                for h in range(H):
                    # start=True clears has_written for the WHOLE bank, so
                    # only the first head of each bank (8 heads/bank) may
                    # set it; later heads' first writes land on cleared
                    # bits and overwrite, the correct first-write behavior.
                    nc.tensor.matmul(
                        ctx_ps[:, h * D:(h + 1) * D],
                        vg[:, h * D:(h + 1) * D],
                        k_sb[:, h * D:(h + 1) * D],
                        start=(nch == 0 and h % 8 == 0),
                        stop=last,
                        skip_group_check=True,
                    )

            for sc in range(SC):
                if 2 <= sc + 1 < SC:  # refill the buffer freed by sc-1
                    nc.sync.dma_start(
                        xpt_t[sc + 1],
                        xpt[(sc + 1) * 128:(sc + 2) * 128],
                    )
                xsc = xpt_t[sc]
                # ---- gate1: hT[m, n] = relu(sum_k w1[k,m].T @ xT[k,n] + b1)
                hT = h_pool.tile([128, 8, NPS], BF16, name="hT", tag="hT")
                for m in range(8):
                    pss = [
                        ps_pool.tile([128, 512], F32, name="g1ps", tag="psA")
                        for _ in range(2)
                    ]
                    for k in range(8):
                        lhs = w1_sb[:, m * 8 + k, :]
                        for half in range(2):
                            nc.tensor.matmul(
                                pss[half],
                                lhs,
                                xsc[:, k, half * 512:(half + 1) * 512],
                                start=(k == 0),
                                stop=(k == 7),
                            )
                    for half in range(2):
                        nc.scalar.activation(
                            hT[:, m, half * 512:(half + 1) * 512],
                            pss[half],
                            AF.Relu,
                            bias=b1_sb[:, m:m + 1],
                        )
                    if sc == 0 and m == 0:
                        # preload the Exp activation table so the softmax
                        # chain doesn't pay the ~1.3us table swap
                        dummy_exp = cpool.tile([1, 16], F32, name="dummy_exp")
                        nc.scalar.activation(
                            dummy_exp, ident_sb[0:1, 0:16], AF.Exp
                        )

                for c in range(8):
                    nch = sc * 8 + c
                    # ---- gate2: g[n, t] = sigmoid(sum_m hT[m,n].T @ w2[m,t])
                    gt = g_pool.tile([128, C], BF16, name="gt", tag="gt")
                    zps = [
                        ps_pool.tile([128, 512], F32, name="g2ps", tag="psA")
                        for _ in range(2)
                    ]
                    for m in range(8):
                        lhs = hT[:, m, c * 128:(c + 1) * 128]
                        for t in range(2):
                            nc.tensor.matmul(
                                zps[t],
                                lhs,
                                w2_sb[:, m, t * 512:(t + 1) * 512],
                                start=(m == 0),
                                stop=(m == 7 and not with_bias),
                            )
                    if with_bias:
                        for t in range(2):
                            nc.tensor.matmul(
                                zps[t],
                                ones_sb,
                                b2_sb[:, t * 512:(t + 1) * 512],
                                start=False,
                                stop=True,
                            )
                    for t in range(2):
                        nc.scalar.activation(
                            gt[:, t * 512:(t + 1) * 512], zps[t], AF.Sigmoid
                        )

                    # ---- kv: [k | v] = sum_k xT[k,n].T @ wkv[k, :],
                    # with the previous chunk's 16 ctx matmuls interleaved
                    # between k-groups (their small weight loads hide under
                    # the 512-wide kv streams; the DVE-produced vg had a
                    # whole gate2 block to land, so the PE never waits)
                    pend = pending_ctx.pop(0) if pending_ctx else None
                    kvps = [
                        ps_pool.tile([128, 512], F32, name="kvps", tag="psA")
                        for _ in range(4)
                    ]
                    for k in range(8):
                        lhs = xsc[:, k, c * 128:(c + 1) * 128]
                        for t in range(4):
                            nc.tensor.matmul(
                                kvps[t],
                                lhs,
                                wkv_sb[:, k, t * 512:(t + 1) * 512],
                                start=(k == 0),
                                stop=(k == 7),
                            )
                        if pend is not None:
                            pk_sb, pvg, pnch = pend
                            for h in (2 * k, 2 * k + 1):
                                nc.tensor.matmul(
                                    ctx_ps[:, h * D:(h + 1) * D],
                                    pvg[:, h * D:(h + 1) * D],
                                    pk_sb[:, h * D:(h + 1) * D],
                                    start=(pnch == 0 and h % 8 == 0),
                                    stop=False,
                                    skip_group_check=True,
                                )
                    k_sb = k_pool.tile([128, C], BF16, name="k_sb", tag="k_sb")
                    nc.scalar.copy(k_sb[:, 0:512], kvps[0])
                    nc.scalar.copy(k_sb[:, 512:1024], kvps[1])
                    vg = vg_pool.tile([128, C], BF16, name="vg", tag="vg")
                    nc.vector.tensor_mul(vg[:, 0:512], kvps[2], gt[:, 0:512])
                    nc.vector.tensor_mul(vg[:, 512:1024], kvps[3], gt[:, 512:1024])
                    pending_ctx.append((k_sb, vg, nch))

            emit_ctx(last=True)

            # =====================================================
            # Softmax over d (free dim of ctxT): st = normalized ctxT.
            # A keep-warm matmul mid-chain stops the HAM re-throttle.
            # =====================================================
            smp = pa.enter_context(tc.tile_pool(name="smpool", bufs=1))
            maxs = smp.tile([64, 16], F32, name="maxs")
            nc.vector.tensor_reduce(
                maxs,
                ctx_ps.rearrange("p (b d) -> p b d", b=16),
                axis=mybir.AxisListType.X,
                op=mybir.AluOpType.max,
            )
            cmx = smp.tile([64, 1024], F32, name="cmx")
            nc.vector.tensor_sub(
                cmx.rearrange("p (h d) -> p h d", h=16),
                ctx_ps.rearrange("p (h d) -> p h d", h=16),
                maxs.unsqueeze(-1).broadcast_to([64, 16, 64]),
            )
            et = smp.tile([64, 1024], F32, name="et")
            nc.scalar.activation(et, cmx, AF.Exp, scale=float(SCALE))
            warm_ps = ps_pool.tile([128, 512], F32, name="warm", tag="psA")
            nc.tensor.matmul(
                warm_ps[0:64, :], et[:, 0:64], et[:, 0:512],
                start=True, stop=True,
            )
            sums = smp.tile([64, 16], F32, name="sums")
            nc.vector.tensor_reduce(
                sums,
                et.rearrange("p (b d) -> p b d", b=16),
                axis=mybir.AxisListType.X,
                op=mybir.AluOpType.add,
            )
            warm1b_ps = ps_pool.tile([128, 512], F32, name="warm1b", tag="psA")
            nc.tensor.matmul(
                warm1b_ps[0:16, :], sums, et[:, 0:512],
                start=True, stop=True,
            )
            recs = smp.tile([64, 16], F32, name="recs")
            nc.vector.reciprocal(recs, sums)
            nc.vector.tensor_mul(
                st.rearrange("p (h d) -> p h d", h=16),
                et.rearrange("p (h d) -> p h d", h=16),
                recs.unsqueeze(-1).broadcast_to([64, 16, 64]),
            )
            warm2_ps = ps_pool.tile([128, 512], F32, name="warm2", tag="psA")
            nc.tensor.matmul(
                warm2_ps[0:64, :], st[:, 0:64], st[:, 0:512],
                start=True, stop=True,
            )

        # =========================================================
        # Phase B, interleaved with block-diag S-pair construction:
        # transposing the pair [ctxT_2j | ctxT_2j+1] ([64, 128]) gives
        # [S_2j over S_2j+1] ([128, 64]); scattered to block-diag, held
        # stationary, and xqT streamed: oT[j*128+e, n] (host transposes).
        # =========================================================
        with ExitStack() as pb:
            smps = pb.enter_context(
                tc.tile_pool(name="smps", bufs=2, space="PSUM")
            )
            oout_pool = pb.enter_context(tc.tile_pool(name="bo", bufs=3))
            bops_pool = pb.enter_context(
                tc.tile_pool(name="bops", bufs=3, space="PSUM")
            )
            # build all 8 spairs back-to-back (PE transposes + tiny DVE
            # copies), then run the 64 output matmuls as one dense stream.
            # PE-mode transposes do NOT count as PE-busy for the HAM clock
            # gate, so small real matmuls are interleaved to keep it warm.
            for j in range(8):
                tp = smps.tile([128, 64], F32, name="smtp", tag="smtp")
                nc.tensor.transpose(
                    tp, st[:, (2 * j) * 64:(2 * j + 2) * 64],
                    ident_sb[0:64, 0:64],
                )
                nc.vector.tensor_copy(spairs[j][0:64, 0:64], tp[0:64, :])
                nc.vector.tensor_copy(spairs[j][64:128, 64:128], tp[64:128, :])
                if j in (2, 5):
                    wtp = smps.tile([128, 64], F32, name="smtp", tag="smtp")
                    nc.tensor.matmul(
                        wtp, spairs[j - 1], spairs[j - 1][:, 0:64],
                        start=True, stop=True,
                    )
            for j in range(8):
                for op2 in range(4):  # 1024-col output groups
                    oout = oout_pool.tile([128, 1024], BF16, name="oo", tag="oo")
                    ops = bops_pool.tile([128, 1024], F32, name="ops", tag="ops")
                    for half in range(2):
                        nc.tensor.matmul(
                            ops[:, half * 512:(half + 1) * 512],
                            spairs[j],
                            xq_sb[j][:, op2 * 1024 + half * 512:
                                     op2 * 1024 + (half + 1) * 512],
                            start=True,
                            stop=True,
                            skip_group_check=True,
                        )
                    if op2 % 2 == 0:
                        nc.vector.tensor_copy(oout, ops)
                    else:
                        nc.scalar.copy(oout, ops)
                    if op2 % 2 == 0:
                        nc.sync.dma_start(
                            o[j * 128:(j + 1) * 128,
                              op2 * 1024:(op2 + 1) * 1024],
                            oout,
                        )
                    else:
                        nc.gpsimd.dma_start(
                            o[j * 128:(j + 1) * 128,
                              op2 * 1024:(op2 + 1) * 1024],
                            oout,
                        )

    nc.compile()
    return nc


def _get_program(with_bias=False):
    key = ("nc", bool(with_bias))
    if key not in _CACHE:
        _CACHE[key] = _build_program(with_bias)
    return _CACHE[key]


def make_in_maps(x1, x2, Wkv1, Wkv2, g1_w1, g1_b1, g1_w2, g1_b2,
                 g2_w1, g2_b1, g2_w2, g2_b2):
    """Core (s, b): cores 0-3 = (s=0, b), cores 4-7 = (s=1, b)."""
    import ml_dtypes
    BF = ml_dtypes.bfloat16
    ident = np.eye(128, dtype=np.float32)
    asf = np.ascontiguousarray

    def proj_chunked(t):
        # [1024, 4096] xT -> [sc*128+p, k, n'] = xT[k*128+p, sc*1024+n']
        return asf(t.reshape(8, 128, SC, NPS).transpose(2, 1, 0, 3)
                   ).reshape(SC * 128, 8, NPS)

    def w1_chunked(w):
        # [1024, 1024] -> [p, m*8+k, m'] = w1[k*128+p, m*128+m']
        return asf(w.astype(BF).reshape(8, 128, 8, 128).transpose(1, 2, 0, 3)
                   ).reshape(128, 64, 128)

    def pk_major(w):
        # [1024, T] -> [p, k, t] = w[k*128+p, t]
        T = w.shape[1]
        return asf(w.astype(BF).reshape(8, 128, T).transpose(1, 0, 2))

    # transposed bf16 activations, computed once per (stream, batch)
    t1 = [asf(x1[b].T).astype(BF) for b in range(x1.shape[0])]
    t2 = [asf(x2[b].T).astype(BF) for b in range(x2.shape[0])]
    t1p = [proj_chunked(t) for t in t1]
    t2p = [proj_chunked(t) for t in t2]
    ws = [
        dict(wkv=pk_major(Wkv1), w1=w1_chunked(g1_w1),
             b1=np.asarray(g1_b1, np.float32), w2=pk_major(g1_w2),
             b2=g1_b2.astype(BF)),
        dict(wkv=pk_major(Wkv2), w1=w1_chunked(g2_w1),
             b1=np.asarray(g2_b1, np.float32), w2=pk_major(g2_w2),
             b2=g2_b2.astype(BF)),
    ]
    in_maps = []
    for core in range(8):
        s, b = core // 4, core % 4
        m = dict(ws[s])
        m["xpt"] = t1p[b] if s == 0 else t2p[b]
        m["xqt"] = t2[b] if s == 0 else t1[b]
        m["ident"] = ident
        in_maps.append(m)
    return in_maps


def kernel(x1, x2, Wkv1, Wkv2, g1_w1, g1_b1, g1_w2, g1_b2,
           g2_w1, g2_b1, g2_w2, g2_b2, _runner=None):
    """Full-input entry point.  Returns (o1, o2), each [4, 4096, 1024] f32."""
    from concourse.bass_utils import run_bass_kernel_spmd

    args = [np.asarray(a, dtype=np.float32) for a in
            (x1, x2, Wkv1, Wkv2, g1_w1, g1_b1, g1_w2, g1_b2,
             g2_w1, g2_b1, g2_w2, g2_b2)]
    with_bias = bool(np.any(args[7]) or np.any(args[11]))  # g1_b2, g2_b2
    nc = _get_program(with_bias)
    in_maps = make_in_maps(*args)
    if _runner is None:
        res = run_bass_kernel_spmd(nc, in_maps, core_ids=list(range(8)))
        results = res.results
    else:
        results = _runner(nc, in_maps)

    B = x1.shape[0]
    o1 = np.empty((B, N, C), dtype=np.float32)
    o2 = np.empty((B, N, C), dtype=np.float32)
    for core in range(8):
        s, b = core // 4, core % 4
        out = np.asarray(results[core]["o"]).astype(np.float32).T  # [N, C]
        if s == 0:
            o2[b] = out   # core projected x1 -> ctx1 -> o2 = q2 @ ctx1
        else:
            o1[b] = out
    return (o1, o2)
